# revision 54
# baseline (speedup 1.0000x reference)
"""Distributed causal multi-head attention for Trainium2 (8 NeuronCores).

Problem: B=8, S=1024, D=768, H=12, DH=64 causal MHA (dense_transformer).
Sharding: pure data parallel — batch element b runs on core b; weights are
replicated. No collectives.

Per-core kernel (bf16 TensorE compute, f32 PSUM accumulation):
  1. Startup is a DMA bandwidth ladder: x rides the otherwise-idle HWDGE
     queue as plain f32 chunk loads (DVE-cast to bf16), W_Q/W_K ride the
     single SWDGE ring as f32->bf16 cast-loads straight to SBUF, and W_V is
     staged bf16 in DRAM then xbar-transpose-loaded. One SWDGE queue makes
     ring service order equal issue order, so each tensor arrives just
     before its consumer. x/wq/wk are transposed to m-major bf16 tiles by
     TensorE (8 transposes share one PSUM tile -> one DVE evict per tile).
     The W_O gather + transpose-loads run mid-attention, gated by a WAW
     hazard on the staging tensor so the scheduler cannot hoist them into
     the startup window (the 8 DMA-completion-semaphore lanes are shared,
     so one slow early DMA stalls unrelated loads).
  2. QKV projections on TensorE -> QT/KT [n,s] (transposed) and V [s,n],
     with a ones-column per head riding V for softmax denominators.
  3. Scores per head pair in alternating PE row-groups (0-63/64-127, the
     pair's matmuls run concurrently per sub-array), causal-exact: key
     block j only computes queries q >= 128j (25% fewer matmul columns and
     exp elements). E_t[j] holds [128, 2L], L = 1024-128j. For j>=4 both
     heads share one PSUM tile and one strided-AP exp. The diagonal
     128-col block is masked post-exp by one gpsimd affine_select.
  4. z^T = V^T E accumulated on TensorE over exact causal ranges
     (partial-column PSUM accumulation); normalize with VectorE
     reciprocal_approx_fast + gpsimd partition_broadcast + multiply.
  5. Output projection: q-blocks 0-3 start during the last pair's z-chains;
     q-blocks 4-7 pre-accumulate their t=0..4 contributions into bf16
     partials as fillers inside pair 5's score loop, leaving one ZT[5]
     matmul + a DVE add per block after the final normalize. Halved output
     DMAs alternate the two HWDGE queues.
"""
import numpy as np

import concourse.bacc as bacc
import concourse.mybir as mybir
import concourse.tile as tile
from concourse.masks import make_identity
from concourse.bass_utils import run_bass_kernel_spmd

f32 = mybir.dt.float32
bf16 = mybir.dt.bfloat16

B = 8
S, D, H, DH = 1024, 768, 12, 64
NT = 6    # n 128-tiles (head pairs)
MT = 6    # m 128-tiles
ST = 8    # s 128-tiles
PC = 2    # p chunks of 512
SCALE = 0.125  # 1/sqrt(DH)
W65 = DH + 1   # per-head V columns incl the ones column

N_CORES = 8


def build(n_cores: int = N_CORES, debug: bool = False):
    # One SWDGE queue: the software-DGE ring then services its DMAs strictly
    # in issue order, which turns scheduler priority into a bandwidth ladder
    # (x cast -> wq cast -> wk -> wv -> wo) instead of fair-share round-robin
    # that starves the startup critical path.
    nc = bacc.Bacc("TRN2", target_bir_lowering=False, debug=False, num_devices=n_cores,
                   num_swdge_queues=1)

    x = nc.dram_tensor("x", [S, D], f32, kind="ExternalInput")
    W_Q = nc.dram_tensor("W_Q", [H, DH, D], f32, kind="ExternalInput")
    W_K = nc.dram_tensor("W_K", [H, DH, D], f32, kind="ExternalInput")
    W_V = nc.dram_tensor("W_V", [H, DH, D], f32, kind="ExternalInput")
    W_O = nc.dram_tensor("W_O", [H, D, DH], f32, kind="ExternalInput")
    out = nc.dram_tensor("out", [S, D], f32, kind="ExternalOutput")

    wvbf = nc.dram_tensor("wvbf", [D, D], bf16)
    wobf = nc.dram_tensor("wobf", [D, D], bf16)   # [m, (i h)]
    warmout = nc.dram_tensor("warmout", [1, 512], f32)

    dbg = {}
    if debug:
        for t in range(NT):
            dbg[f"dZT{t}"] = nc.dram_tensor(f"dZT{t}", [128, S], f32, kind="ExternalOutput")
            dbg[f"dQT{t}"] = nc.dram_tensor(f"dQT{t}", [128, S], f32, kind="ExternalOutput")
            dbg[f"dKT{t}"] = nc.dram_tensor(f"dKT{t}", [128, S], f32, kind="ExternalOutput")
        for j in range(ST):
            dbg[f"dV{j}"] = nc.dram_tensor(f"dV{j}", [128, H * W65], f32, kind="ExternalOutput")

    with tile.TileContext(nc) as tc:
        from contextlib import ExitStack
        with ExitStack() as ctx:
            persist = ctx.enter_context(tc.tile_pool(name="persist", bufs=1))
            epool = ctx.enter_context(tc.tile_pool(name="epool", bufs=2))
            xstage = ctx.enter_context(tc.tile_pool(name="xstage", bufs=4))
            outsb_pool = ctx.enter_context(tc.tile_pool(name="outsb", bufs=4))
            small = ctx.enter_context(tc.tile_pool(name="small", bufs=2))
            ps_mm = ctx.enter_context(tc.tile_pool(name="ps_mm", bufs=2, space="PSUM"))
            ps_sc = ctx.enter_context(tc.tile_pool(name="ps_sc", bufs=2, space="PSUM"))
            ps_zt = ctx.enter_context(tc.tile_pool(name="ps_zt", bufs=2, space="PSUM"))


            # gpsimd init ops first (identity, ones) so the SWDGE cast
            # slot-waits don't delay them; then cast issues; transposes after.
            ident = persist.tile([128, 128], bf16, tag="ident", name="ident")
            make_identity(nc, ident[:])
            warm_src = persist.tile([128, 512], bf16, tag="warm", name="warm")
            nc.gpsimd.memset(warm_src[:], 1.0)
            V_sb = [persist.tile([128, H * W65], bf16, tag=f"V{j}", name=f"V{j}") for j in range(ST)]
            for j in range(ST):
                ones_view = V_sb[j][:].rearrange("p (i w) -> p i w", w=W65)[:, :, DH:W65]
                nc.gpsimd.memset(ones_view, 1.0)

            wqT = [persist.tile([128, D], bf16, tag=f"wqT{m}", name=f"wqT{m}") for m in range(MT)]
            wkT = [persist.tile([128, D], bf16, tag=f"wkT{m}", name=f"wkT{m}") for m in range(MT)]
            wvT = [persist.tile([128, D], bf16, tag=f"wvT{m}", name=f"wvT{m}") for m in range(MT)]
            woT = [persist.tile([128, D], bf16, tag=f"woT{t}", name=f"woT{t}") for t in range(NT)]

            # ---- x / W_Q / W_K: fast plain f32 HWDGE chunk loads, DVE cast
            # to persistent bf16 staging, PE transpose. Plain big HWDGE loads
            # sustain ~350-400 GB/s vs ~200 for SWDGE cast DMAs, and the PE
            # is idle at startup anyway. Only W_V keeps the SWDGE
            # cast-to-DRAM + xbar-transpose-load path (its transposes would
            # not fit the PE budget before z needs it); W_O follows later.
            xT = [persist.tile([128, S], bf16, tag=f"xT{m}", name=f"xT{m}") for m in range(MT)]
            xs0 = xstage.tile([128, 4 * D], bf16, tag="xs0", name="xs0", bufs=1)
            xs1 = xstage.tile([128, 4 * D], bf16, tag="xs1", name="xs1", bufs=1)
            wqs = xstage.tile([128, MT * D], bf16, tag="wqs", name="wqs", bufs=1)
            wks = xstage.tile([128, MT * D], bf16, tag="wks", name="wks", bufs=1)
            x_j = x.ap().rearrange("(j p) m -> p j m", p=128)
            wq_r = W_Q.ap().rearrange("i h m -> (i h) m").rearrange(
                "(r p) m -> p r m", p=128)
            wk_r = W_K.ap().rearrange("i h m -> (i h) m").rearrange(
                "(r p) m -> p r m", p=128)
            # x rides the otherwise-idle HWDGE queue as plain f32 chunk loads
            # with DVE casts; W_Q/W_K ride the single SWDGE ring as cast-loads
            # straight to bf16 SBUF (ring order = priority ladder).
            chunk_plan = (
                [(xs0[:, c * 2 * D:(c + 1) * 2 * D], x_j[:, 2 * c:2 * c + 2, :])
                 for c in range(2)] +
                [(xs1[:, c * 2 * D:(c + 1) * 2 * D], x_j[:, 4 + 2 * c:6 + 2 * c, :])
                 for c in range(2)]
            )
            with tc.high_priority():
                for dst, src in chunk_plan:
                    f = xstage.tile([128, 2 * D], f32, tag="fst", name="fst",
                                    bufs=3)
                    nc.sync.dma_start(
                        f[:].rearrange("p (j m) -> p j m", m=D), src)
                    nc.vector.tensor_copy(dst, f[:])
                nc.gpsimd.dma_start(
                    wqs[:].rearrange("p (r m) -> p r m", m=D), wq_r[:])
                nc.gpsimd.dma_start(
                    wks[:].rearrange("p (r m) -> p r m", m=D), wk_r[:])

            def xrow(j):
                t_ = xs0 if j < 4 else xs1
                return t_[:, (j % 4) * D:(j % 4) * D + D]

            def wqrow(r):
                return wqs[:, r * D:(r + 1) * D]

            def wkrow(r):
                return wks[:, r * D:(r + 1) * D]

            # W_V staging cast rides the SWDGE ring from t=0 (concurrent with
            # the HWDGE loads above); its transpose-loads follow on sync.
            nc.gpsimd.dma_start(wvbf.ap(), W_V.ap().rearrange("i h m -> (i h) m"))
            for m in range(MT):
                sl = slice(m * 128, (m + 1) * 128)
                nc.sync.dma_start(wvT[m][:], wvbf.ap()[:, sl], transpose=True)

            def emit_wo_path():
                # The 9216-descriptor gather is slow; run mid-t-loop so it
                # neither steals startup bandwidth nor stalls unrelated loads
                # via the 8-lane DMA-completion-semaphore round-robin. The
                # scheduler hoists ready DMAs regardless of priority, so gate
                # it with a real WAW hazard: a tiny wobf write sourced from
                # KT[1] (only available once the startup chains ran).
                gate = small.tile([1, 8], bf16, tag="wog", name="wog")
                nc.vector.tensor_copy(gate[:], KT[1][0:1, 0:8])
                nc.gpsimd.dma_start(wobf.ap()[0:1, 0:8], gate[:])
                nc.gpsimd.dma_start(wobf.ap(), W_O.ap().rearrange("i m h -> m i h"))
                for m in range(MT):
                    sl = slice(m * 128, (m + 1) * 128)
                    nc.sync.dma_start(woT[m][:], wobf.ap()[:, sl], transpose=True)

            def emit_x_transposes():
                # bf16 transposes: 8 share one PSUM tile (1 bank, same byte
                # footprint as the f32 [128,512] "mm" tiles) -> one DVE evict
                # fills a whole xT[m].
                for m in range(MT):
                    pt = ps_mm.tile([128, 1024], bf16, tag="mm", name="mm")
                    for j in range(ST):
                        nc.tensor.transpose(
                            pt[:, j * 128:(j + 1) * 128],
                            xrow(j)[:, m * 128:(m + 1) * 128],
                            ident[:])
                    nc.vector.tensor_copy(xT[m][:], pt[:])

            def emit_w_transposes(row_fn, dstT):
                for m in range(MT):
                    pt = ps_mm.tile([128, 1024], bf16, tag="mm", name="mm")
                    for r in range(MT):
                        nc.tensor.transpose(
                            pt[:, r * 128:(r + 1) * 128],
                            row_fn(r)[:, m * 128:(m + 1) * 128],
                            ident[:])
                    nc.vector.tensor_copy(dstT[m][:], pt[:, 0:D])

            QT = [persist.tile([128, S], bf16, tag=f"QT{t}", name=f"QT{t}") for t in range(NT)]
            KT = [persist.tile([128, S], bf16, tag=f"KT{t}", name=f"KT{t}") for t in range(NT)]
            ZT = [persist.tile([128, S], bf16, tag=f"ZT{t}", name=f"ZT{t}") for t in range(NT)]

            def emit_v_tile(j):
                for c2 in range(2):  # n chunks of 384
                    pv = ps_mm.tile([128, 512], f32, tag="mm", name="mm")
                    for m in range(MT):
                        nc.tensor.matmul(
                            pv[:, 0:384],
                            xT[m][:, j * 128:(j + 1) * 128],
                            wvT[m][:, c2 * 384:(c2 + 1) * 384],
                            start=(m == 0), stop=(m == MT - 1),
                        )
                    dst = V_sb[j][:].rearrange("p (i w) -> p i w", w=W65)[:, c2 * 6:(c2 + 1) * 6, 0:DH]
                    src = pv[:, 0:384].rearrange("p (i w) -> p i w", w=DH)
                    nc.vector.tensor_copy(dst, src)

            def emit_qkt_chain(t, which, c):
                dstT, wT = ((QT, wqT) if which == 0 else (KT, wkT))
                pq = ps_mm.tile([128, 512], f32, tag="mm", name="mm")
                for m in range(MT):
                    nc.tensor.matmul(
                        pq[:],
                        wT[m][:, t * 128:(t + 1) * 128],
                        xT[m][:, c * 512:(c + 1) * 512],
                        start=(m == 0), stop=(m == MT - 1),
                    )
                nc.vector.tensor_copy(dstT[t][:, c * 512:(c + 1) * 512], pq[:])

            def emit_score_pair(t, E_t, j):
                # Two heads of the pair in alternating row-groups (0-63 /
                # 64-127) so the PE runs them concurrently per sub-array.
                # Causal-exact ranges: key-block j only attends queries
                # q in [128j, 1024); E_t[j] holds [128, 2L] with L = 1024-128j
                # (head y at cols [y*L, (y+1)*L), tile col c = q - 128j).
                L = 1024 - 128 * j
                if j <= 3:
                    scs = []
                    for y in range(2):
                        hb = 64 * y
                        lhsT = KT[t][hb:hb + 64, j * 128:(j + 1) * 128]
                        sc = ps_sc.tile([128, 1024], f32, tag="sc", name="sc")
                        scs.append(sc)
                        nc.tensor.matmul(sc[:, 0:512], lhsT,
                                         QT[t][hb:hb + 64, 128 * j:128 * j + 512],
                                         start=True, stop=True)
                    for y in range(2):
                        hb = 64 * y
                        lhsT = KT[t][hb:hb + 64, j * 128:(j + 1) * 128]
                        nc.tensor.matmul(scs[y][:, 512:L], lhsT,
                                         QT[t][hb:hb + 64, 128 * j + 512:1024],
                                         start=True, stop=True)
                    for y in range(2):
                        nc.scalar.activation(
                            E_t[j][:, y * L:(y + 1) * L], scs[y][:, 0:L],
                            mybir.ActivationFunctionType.Exp, scale=SCALE)
                        dslice = E_t[j][:, y * L:y * L + 128]
                        nc.gpsimd.affine_select(
                            out=dslice, in_=dslice,
                            compare_op=mybir.AluOpType.is_ge,
                            fill=0.0, base=0,
                            pattern=[[1, 128]], channel_multiplier=-1,
                        )
                else:
                    # One shared PSUM tile (y0 -> cols 0:L, y1 -> 512:512+L so
                    # each head stays inside one PSUM bank) and one exp whose
                    # strided source packs both heads into E contiguously.
                    sc = ps_sc.tile([128, 1024], f32, tag="sc", name="sc")
                    for y in range(2):
                        hb = 64 * y
                        lhsT = KT[t][hb:hb + 64, j * 128:(j + 1) * 128]
                        nc.tensor.matmul(sc[:, y * 512:y * 512 + L], lhsT,
                                         QT[t][hb:hb + 64, 128 * j:1024],
                                         start=True, stop=True)
                    nc.scalar.activation(
                        E_t[j][:, 0:2 * L].rearrange("p (y c) -> p y c", c=L),
                        sc[:].rearrange("p (y c) -> p y c", c=512)[:, :, 0:L],
                        mybir.ActivationFunctionType.Exp, scale=SCALE)
                    dslice = E_t[j][:, 0:2 * L].rearrange(
                        "p (y c) -> p y c", c=L)[:, :, 0:128]
                    nc.gpsimd.affine_select(
                        out=dslice, in_=dslice,
                        compare_op=mybir.AluOpType.is_ge,
                        fill=0.0, base=0,
                        pattern=[[0, 2], [1, 128]], channel_multiplier=-1,
                    )

            def emit_z_chain(t, E_t, c, y):
                jmax = 4 * c + 3
                i = 2 * t + y
                zt = ps_zt.tile([128, 512], f32, tag="zt", name="zt")
                for j in range(jmax + 1):
                    L = 1024 - 128 * j
                    q_lo = max(512 * c, 128 * j)
                    n = 512 * (c + 1) - q_lo
                    e_lo = y * L + (q_lo - 128 * j)
                    p_lo = q_lo - 512 * c
                    nc.tensor.matmul(
                        zt[0:65, p_lo:512],
                        V_sb[j][:, i * W65:(i + 1) * W65],
                        E_t[j][:, e_lo:e_lo + n],
                        start=(j == 0), stop=(j == jmax),
                    )
                den = small.tile([1, 512], f32, tag="den", name="den")
                if t == NT - 1 and c == 1:
                    # last chains: all exps are done, so ACT is idle — pull
                    # the den copy off the congested DVE queue to cut the
                    # final normalize latency chain.
                    nc.scalar.copy(den[:], zt[64:65, :])
                else:
                    nc.vector.tensor_copy(den[:], zt[64:65, :])
                nc.vector.reciprocal_approx_fast(den[:], den[:])
                bc = small.tile([64, 512], f32, tag="bc", name="bc")
                nc.gpsimd.partition_broadcast(bc[:], den[:])
                nc.vector.tensor_mul(
                    ZT[t][64 * y:64 * y + 64, c * 512:(c + 1) * 512],
                    zt[0:64, :], bc[:])

            # ---- schedule ----
            # HAM warm-up: dense dummy matmuls while startup DMAs stream.
            # PE-transpose-mode ops don't count as PE-busy for the HAM clock
            # gate, so without these the whole QKV phase runs at 1.2 GHz.
            warm_ps = ps_zt.tile([128, 512], f32, tag="zt", name="zt")
            NWARM = 32
            for w in range(NWARM):
                nc.tensor.matmul(warm_ps[:], warm_src[:, 0:128], warm_src[:],
                                 start=(w == 0), stop=(w == NWARM - 1))
            warm_out = small.tile([1, 512], f32, tag="den", name="den")
            nc.vector.tensor_copy(warm_out[:], warm_ps[0:1, :])
            nc.sync.dma_start(warmout.ap(), warm_out[:])

            emit_x_transposes()
            emit_w_transposes(wqrow, wqT)
            emit_w_transposes(wkrow, wkT)
            for w in range(2):
                for c in range(PC):
                    emit_qkt_chain(0, w, c)
            for w in range(2):
                for c in range(PC):
                    emit_qkt_chain(1, w, c)

            def alloc_E():
                return [epool.tile([128, 2 * (1024 - 128 * j)], bf16,
                                   tag=f"E{j}", name=f"E{j}")
                        for j in range(ST)]

            def emit_out_proj(qj):
                osb = outsb_pool.tile([128, D], f32, tag="osb", name="osb")
                for mc in range(2):
                    po = ps_mm.tile([128, 512], f32, tag="mm", name="mm")
                    for tt in range(NT):
                        nc.tensor.matmul(
                            po[:, 0:384],
                            ZT[tt][:, qj * 128:(qj + 1) * 128],
                            woT[tt][:, mc * 384:(mc + 1) * 384],
                            start=(tt == 0), stop=(tt == NT - 1),
                        )
                    nc.vector.tensor_copy(osb[:, mc * 384:(mc + 1) * 384],
                                          po[:, 0:384])
                    # drain each 384-col half as soon as it is evicted,
                    # alternating HWDGE queues so the writes go out 2-wide
                    q = nc.sync if (2 * qj + mc) % 2 == 0 else nc.scalar
                    q.dma_start(
                        out.ap()[qj * 128:(qj + 1) * 128,
                                 mc * 384:(mc + 1) * 384],
                        osb[:, mc * 384:(mc + 1) * 384])

            # For q-blocks gated on the very last z-chains (qj 4-7), the
            # t=0..4 part of the projection runs early (fillers in pair 5's
            # score loop, bf16 partials); only one ZT[5] matmul + a DVE add
            # remain after the final normalize.
            OP = {qj: persist.tile([128, D], bf16, tag=f"OP{qj}", name=f"OP{qj}")
                  for qj in range(4, ST)}

            def emit_out_partial(qj):
                for mc in range(2):
                    po = ps_mm.tile([128, 512], f32, tag="mm", name="mm")
                    for tt in range(NT - 1):
                        nc.tensor.matmul(
                            po[:, 0:384],
                            ZT[tt][:, qj * 128:(qj + 1) * 128],
                            woT[tt][:, mc * 384:(mc + 1) * 384],
                            start=(tt == 0), stop=(tt == NT - 2),
                        )
                    nc.vector.tensor_copy(
                        OP[qj][:, mc * 384:(mc + 1) * 384], po[:, 0:384])

            def emit_out_final(qj):
                osb = outsb_pool.tile([128, D], f32, tag="osb", name="osb")
                for mc in range(2):
                    po = ps_mm.tile([128, 512], f32, tag="mm", name="mm")
                    nc.tensor.matmul(
                        po[:, 0:384],
                        ZT[NT - 1][:, qj * 128:(qj + 1) * 128],
                        woT[NT - 1][:, mc * 384:(mc + 1) * 384],
                        start=True, stop=True,
                    )
                    nc.vector.tensor_add(
                        osb[:, mc * 384:(mc + 1) * 384], po[:, 0:384],
                        OP[qj][:, mc * 384:(mc + 1) * 384])
                    q = nc.sync if (2 * qj + mc) % 2 == 0 else nc.scalar
                    q.dma_start(
                        out.ap()[qj * 128:(qj + 1) * 128,
                                 mc * 384:(mc + 1) * 384],
                        osb[:, mc * 384:(mc + 1) * 384])

            E_tiles = {0: alloc_E()}
            start_j = 0
            for t in range(NT):
                if t == 1:
                    emit_wo_path()
                E_t = E_tiles[t]
                fillers = []
                if t == 0:
                    fillers += [lambda j=j: emit_v_tile(j) for j in range(ST)]
                elif t + 1 < NT:
                    fillers += [lambda w=w, c=c, tt=t + 1: emit_qkt_chain(tt, w, c)
                                for w in range(2) for c in range(PC)]
                else:
                    fillers += [lambda qj=qj: emit_out_partial(qj)
                                for qj in range(4, ST)]
                    fillers += [lambda qj=qj: emit_out_proj(qj)
                                for qj in range(2)]
                fi = 0
                for j in range(start_j, ST):
                    emit_score_pair(t, E_t, j)
                    if j == 4:
                        emit_z_chain(t, E_t, 0, 0)
                    elif j == 5:
                        emit_z_chain(t, E_t, 0, 1)
                    if fi < len(fillers):
                        fillers[fi]()
                        fi += 1
                while fi < len(fillers):
                    fillers[fi]()
                    fi += 1
                # prefetch pair t+1's first three score tiles so the PE has
                # work while ACT drains this pair's last exps before z(c1)
                if t + 1 < NT:
                    E_tiles[t + 1] = alloc_E()
                    emit_score_pair(t + 1, E_tiles[t + 1], 0)
                    emit_score_pair(t + 1, E_tiles[t + 1], 1)
                    emit_score_pair(t + 1, E_tiles[t + 1], 2)
                    start_j = 3
                for y in range(2):
                    emit_z_chain(t, E_t, 1, y)

            # ---- output projection: the c=1 normalize latency is hidden by
            # the remaining first-half projections, then only the ZT[5]
            # matmul + DVE add per q-block stand between the last z and out.
            for qj in range(2, 4):
                emit_out_proj(qj)
            for qj in range(4, ST):
                emit_out_final(qj)

            if debug:
                dpool = ctx.enter_context(tc.tile_pool(name="dpool", bufs=2))

                def dump(name, tile_ap):
                    fs = 1
                    for s_ in tile_ap.shape[1:]:
                        fs *= s_
                    f = dpool.tile([128, fs], f32, tag="d", name="d")
                    nc.vector.tensor_copy(f[:, 0:fs], tile_ap)
                    nc.sync.dma_start(dbg[name].ap(), f[:, 0:fs])

                for t in range(NT):
                    dump(f"dZT{t}", ZT[t][:])
                    dump(f"dQT{t}", QT[t][:])
                    dump(f"dKT{t}", KT[t][:])
                for j in range(ST):
                    dump(f"dV{j}", V_sb[j][:])

    nc.compile()
    return nc


_NC_CACHE = None


def _get_nc():
    global _NC_CACHE
    if _NC_CACHE is None:
        _NC_CACHE = build(N_CORES)
    return _NC_CACHE


def run(inputs, trace=False, **kwargs):
    nc = _get_nc()
    weights = {k: np.ascontiguousarray(np.asarray(inputs[k], dtype=np.float32))
               for k in ("W_Q", "W_K", "W_V", "W_O")}
    xs = np.ascontiguousarray(np.asarray(inputs["x"], dtype=np.float32))
    in_maps = [dict(weights, x=xs[b]) for b in range(B)]
    res = run_bass_kernel_spmd(nc, in_maps, core_ids=list(range(N_CORES)),
                               trace=trace, **kwargs)
    out = np.stack([np.asarray(res.results[b]["out"]) for b in range(B)], axis=0)
    return out.astype(np.float32), res


def kernel(**inputs) -> np.ndarray:
    out, _ = run(inputs, trace=False)
    return out

